# revision 5
# baseline (speedup 1.0000x reference)
"""Trainium2 Bass kernel for nn_Cate2Classifier (8 NeuronCores, data-parallel over batch).

Pipeline per core (32 of 256 samples):
  embedding gather (indirect DMA, bf16 pre-scaled x64)
  -> DMA-transpose (XBAR) to channel-major, DVE convert to fp8e4
  -> conv1d k=3/k=5 GLU branches as fp8 DoubleRow matmuls (K=256 per MM)
  -> max-pool over sequence -> BatchNorm1 (cross-core moment AllReduce) -> FC
  -> BatchNorm2 (AllReduce) -> ReLU -> classifier -> per-sample column mask to -100.

Scaling: emb and conv weights are quantized to fp8 with a x64 scale each, so conv
PSUM values are 4096x the true pre-activations.  The gate branch is descaled
exactly inside the sigmoid (activation scale=2^-12); the linear branch keeps the
x4096 scale (bias pre-scaled on host) which BatchNorm1 absorbs (its stats are
descaled before EPS is applied, keeping numerics exact).

Layout: activations live as [128 chan-part, 4 chunk-planes, packed positions] fp8
tiles; each sample's sequence is padded with 2 zero cols each side so conv taps
read only that sample's window; one DoubleRow matmul streams 512 positions and
contracts 256 channels (2 planes).
"""
import os
import numpy as np
import ml_dtypes

import concourse.bass as bass
import concourse.mybir as mybir
import concourse.tile as tile
import bass_rust
from concourse.bass_utils import run_bass_kernel_spmd
from concourse.masks import make_identity

P = 128
NCORES = 8
B, BS = 256, 32          # batch, batch per core
Lt, Ld = 60, 300         # title/desc lengths
V, D, H, NCLS = 100000, 512, 1024, 135
NC1, M = 10, 20
EPS = 1e-5

SCALE = 64.0             # fp8 quant scale for emb and conv weights
SCALE2 = SCALE * SCALE   # 4096: scale of conv PSUM outputs
INV_SCALE2 = 1.0 / SCALE2

ST, SD = 64, 304                         # per-sample padded strides
WT = 2 + BS * ST + 2                     # 2052 packed title cols (+global margins)
WD = 2 + BS * SD + 2                     # 9732 packed desc cols
WT_PAD = ((WT + 127) // 128) * 128       # 2176
WD_PAD = ((WD + 127) // 128) * 128       # 9856
NT_TILES = WT_PAD // 128                 # 17
ND_TILES = WD_PAD // 128                 # 77
NTOK = NT_TILES + ND_TILES               # 94 gather tiles of 128 tokens

NBLK_T = -(-(2 + ST * (BS - 1) + Lt) // 512)   # 4 blocks cover title data
NBLK_D = -(-(2 + SD * (BS - 1) + Ld) // 512)   # 19 blocks cover desc data

XW = 528                                 # X tile plane stride (>=516, mult of 16)

f32 = mybir.dt.float32
bf16 = mybir.dt.bfloat16
f8 = mybir.dt.float8e4
i32 = mybir.dt.int32
i8 = mybir.dt.int8
DR = mybir.MatmulPerfMode.DoubleRow

_WAIT_CAP = 1  # walrus rejects >1 sync wait per instruction


def _legalize_waits(nc, cap=_WAIT_CAP):
    """Split instructions with too many sync waits into preceding same-engine Drains."""
    n_added = 0
    for fn in nc.m.functions:
        for bb in fn.blocks:
            new_list = []
            changed = False
            for inst in bb.instructions:
                si = inst.sync_info
                waits = list(si.on_wait) if si is not None else []
                if len(waits) > cap:
                    changed = True
                    extra, keep = waits[:-cap], waits[-cap:]
                    while extra:
                        chunk, extra = extra[:cap], extra[cap:]
                        d = mybir.InstDrain(
                            name=f"I-waitsplit-{n_added}", engine=inst.engine
                        )
                        d.sync_info = bass_rust.SyncInfo(on_wait=chunk, on_update=[])
                        nc.register_instruction(d)
                        new_list.append(d)
                        n_added += 1
                    inst.sync_info = bass_rust.SyncInfo(
                        on_wait=keep, on_update=list(si.on_update)
                    )
                new_list.append(inst)
            if changed:
                bb.instructions = new_list
    return n_added


def _build():
    nc = bass.Bass(num_devices=NCORES, num_swdge_queues=int(os.environ.get("K_SWQ", "4")))

    emb_d = nc.dram_tensor("emb", [V, D], bf16, kind="ExternalInput")
    c3w_d = nc.dram_tensor("c3w", [P, 96, P], f8, kind="ExternalInput")
    c5w_d = nc.dram_tensor("c5w", [P, 160, P], f8, kind="ExternalInput")
    fcw_d = nc.dram_tensor("fcw", [P, P, P], bf16, kind="ExternalInput")  # [p, 16*8, 128]
    clfw_d = nc.dram_tensor("clfw", [P, 8, NCLS], bf16, kind="ExternalInput")
    c3b_d = nc.dram_tensor("c3b", [P, 8], f32, kind="ExternalInput")
    c5b_d = nc.dram_tensor("c5b", [P, 8], f32, kind="ExternalInput")
    bn1g_d = nc.dram_tensor("bn1g", [P, 16], f32, kind="ExternalInput")
    bn1b_d = nc.dram_tensor("bn1b", [P, 16], f32, kind="ExternalInput")
    bn2g_d = nc.dram_tensor("bn2g", [P, 8], f32, kind="ExternalInput")
    bn2b_d = nc.dram_tensor("bn2b", [P, 8], f32, kind="ExternalInput")
    fcb_d = nc.dram_tensor("fcb", [P, 8], f32, kind="ExternalInput")
    clfb_d = nc.dram_tensor("clfb", [P, 2], f32, kind="ExternalInput")
    tok_d = nc.dram_tensor("tok", [P, NTOK], i32, kind="ExternalInput")
    cate1_d = nc.dram_tensor("cate1", [BS, 1], i32, kind="ExternalInput")
    mask1_d = nc.dram_tensor("mask1", [NC1, M], i32, kind="ExternalInput")
    out_d = nc.dram_tensor("out", [BS, NCLS], f32, kind="ExternalOutput")

    import contextlib
    loop_n = int(os.environ.get("K_LOOP", "0"))
    tr_pe = bool(os.environ.get("K_TR_PE"))   # fallback: transposes on PE
    with tile.TileContext(nc, num_cores=NCORES) as tc:
        with tc.tile_pool(name="const", bufs=1) as cp, (
            tc.For_i(0, loop_n, 1) if loop_n else contextlib.nullcontext()
        ):
            identf = cp.tile([P, P], f32, tag="identf")
            make_identity(nc, identf)
            tok_sb = cp.tile([P, NTOK], i32, tag="tok")
            nc.gpsimd.dma_start(tok_sb[:], tok_d[:])
            c3b = cp.tile([P, 8], f32, tag="c3b")
            nc.gpsimd.dma_start(c3b[:], c3b_d[:])
            c5b = cp.tile([P, 8], f32, tag="c5b")
            nc.gpsimd.dma_start(c5b[:], c5b_d[:])
            bn1g = cp.tile([P, 16], f32, tag="bn1g")
            nc.gpsimd.dma_start(bn1g[:], bn1g_d[:])
            bn1b = cp.tile([P, 16], f32, tag="bn1b")
            nc.gpsimd.dma_start(bn1b[:], bn1b_d[:])
            bn2g = cp.tile([P, 8], f32, tag="bn2g")
            nc.gpsimd.dma_start(bn2g[:], bn2g_d[:])
            bn2b = cp.tile([P, 8], f32, tag="bn2b")
            nc.gpsimd.dma_start(bn2b[:], bn2b_d[:])
            fcb = cp.tile([P, 8], f32, tag="fcb")
            nc.gpsimd.dma_start(fcb[:], fcb_d[:])
            clfb = cp.tile([P, 2], f32, tag="clfb")
            nc.gpsimd.dma_start(clfb[:], clfb_d[:])
            epst = cp.tile([P, 1], f32, tag="epst")
            nc.vector.memset(epst[:], EPS)
            if tr_pe:
                identb = cp.tile([P, P], bf16, tag="identb")
                make_identity(nc, identb)

            # running max accumulator: [P, 16*BS], chunk c = branch*4 + oj,
            # branch order [t1, d1, t2, d2]
            rmax = cp.tile([P, 16 * BS], f32, tag="rmax", name="rmax")
            nc.vector.memset(rmax[:], -1e30)

            with (
                tc.tile_pool(name="wconv", bufs=1) as wp,
                tc.tile_pool(name="xbuf", bufs=1) as xp,
            ):
                c3w = wp.tile([P, 96, P], f8, tag="c3w")
                nc.gpsimd.dma_start(c3w[:], c3w_d[:])
                c5w = wp.tile([P, 160, P], f8, tag="c5w")
                nc.gpsimd.dma_start(c5w[:], c5w_d[:])

                # per-block activation tiles: tile n covers packed cols
                # [512n, 512n+516) (+4-col halo); plane dim = 4 channel chunks
                xtb = [xp.tile([P, 4, XW], f8, tag=f"xtb{n}", name=f"xtb{n}")
                       for n in range(NBLK_T)]
                xdb = [xp.tile([P, 4, XW], f8, tag=f"xdb{n}", name=f"xdb{n}")
                       for n in range(NBLK_D)]

                def block_spans(c0, nblk):
                    out = []
                    for n in range(max(0, (c0 - 515) // 512), nblk):
                        lo, hi = max(c0, 512 * n), min(c0 + P, 512 * n + 516)
                        if lo < hi:
                            out.append((n, lo, hi))
                        if 512 * n > c0 + P:
                            break
                    return out

                with (
                    tc.tile_pool(name="gst", bufs=6) as gst,
                    tc.tile_pool(name="tst", bufs=10) as tstp,
                    tc.tile_pool(name="gps", bufs=2, space="PSUM") as gps,
                    tc.tile_pool(name="cps", bufs=3, space="PSUM") as cps,
                    tc.tile_pool(name="csb", bufs=3) as csb,
                    tc.tile_pool(name="red", bufs=4) as red,
                ):
                    for j in range(NTOK):
                        gath = gst.tile([P, D], bf16, tag="gath")
                        nc.gpsimd.indirect_dma_start(
                            out=gath[:], out_offset=None,
                            in_=emb_d[:],
                            in_offset=bass.IndirectOffsetOnAxis(
                                ap=tok_sb[:, j:j + 1], axis=0
                            ),
                        )
                        if j < NT_TILES:
                            dst, c0, nblk = xtb, j * P, NBLK_T
                        else:
                            dst, c0, nblk = xdb, (j - NT_TILES) * P, NBLK_D
                        spans = block_spans(c0, nblk)
                        for ci in range(4):
                            if tr_pe:
                                tps = gps.tile([P, P], bf16, tag="tps")
                                nc.tensor.transpose(
                                    out=tps[:], in_=gath[:, ci * P:(ci + 1) * P],
                                    identity=identb[:])
                                srcv = tps
                            else:
                                tst = tstp.tile([P, P], bf16, tag="tst")
                                eng = nc.sync if ci % 2 == 0 else nc.scalar
                                eng.dma_start_transpose(
                                    out=tst[:], in_=gath[:, ci * P:(ci + 1) * P])
                                srcv = tst
                            for n, lo, hi in spans:
                                nc.vector.tensor_copy(
                                    out=dst[n][:, ci, lo - 512 * n:hi - 512 * n],
                                    in_=srcv[:, lo - c0:hi - c0],
                                )

                    def conv_branches(Xb, nblk, stride, L, br3, br5):
                        # sample s data occupies virtual cols [2+stride*s, 2+stride*s+L)
                        for n in range(nblk):
                            v0 = n * 512
                            s_lo = max(0, (v0 - 2 - L + 1) // stride)
                            s_hi = min(BS - 1, (v0 + 511 - 2) // stride)
                            segs = []
                            for s in range(s_lo, s_hi + 1):
                                a = max(2 + stride * s, v0)
                                b = min(2 + stride * s + L, v0 + 512)
                                if a < b:
                                    segs.append((s, a - v0, b - v0))
                            for K, wsb, bsb, br in (
                                (3, c3w, c3b, br3), (5, c5w, c5b, br5),
                            ):
                                pad = (K - 1) // 2
                                for oj in range(4):
                                    psa = cps.tile([P, 512], f32, tag="psa")
                                    psg = cps.tile([P, 512], f32, tag="psg")
                                    nmm = 2 * K
                                    for half, ps in ((0, psa), (1, psg)):
                                        ojj = oj + 4 * half
                                        i = 0
                                        for k in range(K):
                                            off = k - pad + 2
                                            for cj in range(2):
                                                i2 = (((k * 2 + cj) * 8) + ojj) * 2
                                                nc.tensor.matmul(
                                                    ps[:],
                                                    wsb[:, i2:i2 + 2, :],
                                                    Xb[n][:, 2 * cj:2 * cj + 2,
                                                          off:off + 512],
                                                    start=(i == 0),
                                                    stop=(i == nmm - 1),
                                                    perf_mode=DR,
                                                )
                                                i += 1
                                    # epilogue: GLU = (psa + ba*4096) * sigmoid(psg/4096 + bg)
                                    sg = csb.tile([P, 512], bf16, tag="sg")
                                    nc.scalar.activation(
                                        out=sg[:], in_=psg[:],
                                        func=mybir.ActivationFunctionType.Sigmoid,
                                        bias=bsb[:, oj + 4:oj + 5],
                                        scale=INV_SCALE2,
                                    )
                                    av = csb.tile([P, 512], bf16, tag="av")
                                    nc.scalar.activation(
                                        out=av[:], in_=psa[:],
                                        func=mybir.ActivationFunctionType.Identity,
                                        bias=bsb[:, oj:oj + 1],
                                    )
                                    glu = csb.tile([P, 512], bf16, tag="glu")
                                    nc.vector.tensor_mul(out=glu[:], in0=av[:], in1=sg[:])
                                    c16 = (br * 4 + oj) * BS
                                    if stride == ST:
                                        # block == 8 whole samples: one 3D reduce
                                        tmp8 = red.tile([P, 8], f32, tag="tmp8")
                                        g3 = glu[:, :].rearrange(
                                            "p (s l) -> p s l", l=ST
                                        )[:, :, 2:2 + L]
                                        nc.vector.tensor_reduce(
                                            out=tmp8[:], in_=g3,
                                            axis=mybir.AxisListType.X,
                                            op=mybir.AluOpType.max,
                                        )
                                        s0 = v0 // ST
                                        nc.vector.tensor_tensor(
                                            out=rmax[:, c16 + s0:c16 + s0 + 8],
                                            in0=rmax[:, c16 + s0:c16 + s0 + 8],
                                            in1=tmp8[:], op=mybir.AluOpType.max,
                                        )
                                    else:
                                        for s, a, b in segs:
                                            tmp1 = red.tile([P, 1], f32, tag="tmp1")
                                            nc.vector.tensor_reduce(
                                                out=tmp1[:], in_=glu[:, a:b],
                                                axis=mybir.AxisListType.X,
                                                op=mybir.AluOpType.max,
                                            )
                                            nc.vector.tensor_tensor(
                                                out=rmax[:, c16 + s:c16 + s + 1],
                                                in0=rmax[:, c16 + s:c16 + s + 1],
                                                in1=tmp1[:], op=mybir.AluOpType.max,
                                            )

                    conv_branches(xtb, NBLK_T, ST, Lt, 0, 2)   # t1, t2
                    conv_branches(xdb, NBLK_D, SD, Ld, 1, 3)   # d1, d2

            # ---- tail: BN1 -> FC -> BN2 -> ReLU -> clf -> mask ----
            with (
                tc.tile_pool(name="tw", bufs=1) as tw,
                tc.tile_pool(name="tps", bufs=1, space="PSUM") as tps,
                tc.tile_pool(name="tsb", bufs=1) as tsb,
                tc.tile_pool(name="dram", bufs=1, space="DRAM") as dp,
            ):
                fcw = tw.tile([P, P, P], bf16, tag="fcw")
                nc.gpsimd.dma_start(fcw[:], fcw_d[:])
                clfw = tw.tile([P, 8, NCLS], bf16, tag="clfw")
                nc.gpsimd.dma_start(clfw[:], clfw_d[:])

                def bn_stats(xall, nch, gam, bet, pre_scale):
                    """Cross-core batch moments + scale/shift from xall [P, nch*BS]
                    whose values are pre_scale times the true ones.
                    Returns (s_apply, t) [P, nch] with s_apply including 1/pre_scale."""
                    dump = tsb.tile([P, nch * BS], f32, tag=f"dump{nch}")
                    nc.vector.tensor_mul(out=dump[:], in0=xall[:], in1=xall[:])
                    mom = tsb.tile([P, 2 * nch], f32, tag=f"mom{nch}")
                    nc.vector.tensor_reduce(
                        out=mom[:, 0:nch],
                        in_=xall[:, :].rearrange("p (c b) -> p c b", b=BS),
                        axis=mybir.AxisListType.X, op=mybir.AluOpType.add,
                    )
                    nc.vector.tensor_reduce(
                        out=mom[:, nch:2 * nch],
                        in_=dump[:, :].rearrange("p (c b) -> p c b", b=BS),
                        axis=mybir.AxisListType.X, op=mybir.AluOpType.add,
                    )
                    cc_in = dp.tile([P, 2 * nch], f32, tag=f"cci{nch}")
                    cc_out = dp.tile([P, 2 * nch], f32, tag=f"cco{nch}")
                    nc.gpsimd.dma_start(cc_in[:], mom[:])
                    nc.gpsimd.collective_compute(
                        "AllReduce", mybir.AluOpType.add,
                        replica_groups=[list(range(NCORES))],
                        ins=[cc_in[:].opt()], outs=[cc_out[:].opt()],
                    )
                    momr = tsb.tile([P, 2 * nch], f32, tag=f"momr{nch}")
                    nc.gpsimd.dma_start(momr[:], cc_out[:])
                    mean = tsb.tile([P, nch], f32, tag=f"mean{nch}")
                    nc.vector.tensor_scalar(
                        out=mean[:], in0=momr[:, 0:nch],
                        scalar1=1.0 / (B * pre_scale),
                        scalar2=None, op0=mybir.AluOpType.mult,
                    )
                    var = tsb.tile([P, nch], f32, tag=f"var{nch}")
                    nc.vector.tensor_scalar(
                        out=var[:], in0=momr[:, nch:2 * nch],
                        scalar1=1.0 / (B * pre_scale * pre_scale),
                        scalar2=None, op0=mybir.AluOpType.mult,
                    )
                    msq = tsb.tile([P, nch], f32, tag=f"msq{nch}")
                    nc.vector.tensor_mul(out=msq[:], in0=mean[:], in1=mean[:])
                    nc.vector.tensor_tensor(
                        out=var[:], in0=var[:], in1=msq[:],
                        op=mybir.AluOpType.subtract,
                    )
                    std = tsb.tile([P, nch], f32, tag=f"std{nch}")
                    nc.scalar.activation(
                        out=std[:], in_=var[:],
                        func=mybir.ActivationFunctionType.Sqrt, bias=epst[:, 0:1],
                    )
                    rstd = tsb.tile([P, nch], f32, tag=f"rstd{nch}")
                    nc.vector.reciprocal(out=rstd[:], in_=std[:])
                    s = tsb.tile([P, nch], f32, tag=f"s{nch}")
                    nc.vector.tensor_mul(out=s[:], in0=rstd[:], in1=gam[:])
                    t = tsb.tile([P, nch], f32, tag=f"t{nch}")
                    nc.vector.tensor_mul(out=t[:], in0=mean[:], in1=s[:])
                    nc.vector.tensor_tensor(
                        out=t[:], in0=bet[:], in1=t[:], op=mybir.AluOpType.subtract,
                    )
                    if pre_scale != 1.0:
                        sa = tsb.tile([P, nch], f32, tag=f"sa{nch}")
                        nc.vector.tensor_scalar(
                            out=sa[:], in0=s[:], scalar1=1.0 / pre_scale,
                            scalar2=None, op0=mybir.AluOpType.mult,
                        )
                        return sa, t
                    return s, t

                s1, t1 = bn_stats(rmax, 16, bn1g, bn1b, SCALE2)
                xn = []
                for c in range(16):
                    x = tsb.tile([P, BS], bf16, tag=f"xn{c}")
                    nc.vector.tensor_scalar(
                        out=x[:], in0=rmax[:, c * BS:(c + 1) * BS],
                        scalar1=s1[:, c:c + 1], scalar2=t1[:, c:c + 1],
                        op0=mybir.AluOpType.mult, op1=mybir.AluOpType.add,
                    )
                    xn.append(x)

                hpre = tsb.tile([P, 8 * BS], f32, tag="hpre")
                for hj in range(8):
                    psh = tps.tile([P, BS], f32, tag="psh")
                    for c in range(16):
                        nc.tensor.matmul(
                            psh[:], fcw[:, c * 8 + hj, :], xn[c][:],
                            start=(c == 0), stop=(c == 15),
                        )
                    nc.vector.tensor_scalar(
                        out=hpre[:, hj * BS:(hj + 1) * BS], in0=psh[:],
                        scalar1=fcb[:, hj:hj + 1],
                        scalar2=None, op0=mybir.AluOpType.add,
                    )

                s2, t2 = bn_stats(hpre, 8, bn2g, bn2b, 1.0)
                hn = []
                for hj in range(8):
                    h = tsb.tile([P, BS], bf16, tag=f"hn{hj}")
                    nc.vector.tensor_scalar(
                        out=h[:], in0=hpre[:, hj * BS:(hj + 1) * BS],
                        scalar1=s2[:, hj:hj + 1], scalar2=t2[:, hj:hj + 1],
                        op0=mybir.AluOpType.mult, op1=mybir.AluOpType.add,
                    )
                    nc.vector.tensor_scalar(
                        out=h[:], in0=h[:], scalar1=0.0, scalar2=None,
                        op0=mybir.AluOpType.max,
                    )
                    hn.append(h)

                psca = tps.tile([P, BS], f32, tag="psca")
                for c in range(8):
                    nc.tensor.matmul(
                        psca[:], clfw[:, c, 0:P], hn[c][:],
                        start=(c == 0), stop=(c == 7),
                    )
                pscb = tps.tile([7, BS], f32, tag="pscb")
                for c in range(8):
                    nc.tensor.matmul(
                        pscb[:], clfw[:, c, P:NCLS], hn[c][:],
                        start=(c == 0), stop=(c == 7),
                    )
                outa = tsb.tile([P, BS], f32, tag="outa")
                nc.vector.tensor_scalar(
                    out=outa[:], in0=psca[:], scalar1=clfb[:, 0:1],
                    scalar2=None, op0=mybir.AluOpType.add,
                )
                outb = tsb.tile([7, BS], f32, tag="outb")
                nc.vector.tensor_scalar(
                    out=outb[:], in0=pscb[:], scalar1=clfb[0:7, 1:2],
                    scalar2=None, op0=mybir.AluOpType.add,
                )
                # transpose to [samples, classes]
                ta = tps.tile([BS, P], f32, tag="ta")
                nc.tensor.transpose(out=ta[:], in_=outa[:], identity=identf[:])
                tb = tps.tile([BS, 7], f32, tag="tb")
                nc.tensor.transpose(out=tb[:], in_=outb[:], identity=identf[0:7, 0:7])
                final = tsb.tile([BS, NCLS], f32, tag="final")
                nc.vector.tensor_copy(out=final[:, 0:P], in_=ta[:])
                nc.vector.tensor_copy(out=final[:, P:NCLS], in_=tb[:])

                # per-sample mask columns -> -100
                cate_sb = tsb.tile([BS, 1], i32, tag="cate")
                nc.gpsimd.dma_start(cate_sb[:], cate1_d[:])
                cols = tsb.tile([BS, M], i32, tag="cols")
                nc.gpsimd.indirect_dma_start(
                    out=cols[:], out_offset=None, in_=mask1_d[:],
                    in_offset=bass.IndirectOffsetOnAxis(ap=cate_sb[:, 0:1], axis=0),
                )
                colsf = tsb.tile([BS, M], f32, tag="colsf")
                nc.vector.tensor_copy(out=colsf[:], in_=cols[:])
                iot = tsb.tile([BS, NCLS], i32, tag="iot")
                nc.gpsimd.iota(iot[:], pattern=[[1, NCLS]], base=0, channel_multiplier=0)
                iof = tsb.tile([BS, NCLS], f32, tag="iof")
                nc.vector.tensor_copy(out=iof[:], in_=iot[:])
                msk = tsb.tile([BS, NCLS], i8, tag="msk")
                mk = tsb.tile([BS, NCLS], i8, tag="mk")
                nc.vector.tensor_scalar(
                    out=msk[:], in0=iof[:], scalar1=colsf[:, 0:1], scalar2=None,
                    op0=mybir.AluOpType.is_equal,
                )
                for k in range(1, M):
                    nc.vector.tensor_scalar(
                        out=mk[:], in0=iof[:], scalar1=colsf[:, k:k + 1], scalar2=None,
                        op0=mybir.AluOpType.is_equal,
                    )
                    nc.vector.tensor_tensor(
                        out=msk[:], in0=msk[:], in1=mk[:], op=mybir.AluOpType.max,
                    )
                neg = tsb.tile([BS, NCLS], f32, tag="neg")
                nc.vector.memset(neg[:], -100.0)
                nc.vector.copy_predicated(out=final[:], mask=msk[:], data=neg[:])
                nc.gpsimd.dma_start(out_d[:], final[:])

    _legalize_waits(nc)
    return nc


_NC_CACHE = None


def _get_nc():
    global _NC_CACHE
    if _NC_CACHE is None:
        _NC_CACHE = _build()
    return _NC_CACHE


def _pack_tokens(mat, L, stride, width_pad):
    """mat: (BS, L) int tokens -> padded packed token-index array of len width_pad."""
    out = np.zeros(width_pad, dtype=np.int32)
    for s in range(BS):
        out[4 + stride * s: 4 + stride * s + L] = mat[s]
    return out


def make_in_maps(title, desc, cate1, mask1, emb, conv3_w, conv3_b, conv5_w, conv5_b,
                 fc_w, fc_b, clf_w, clf_b, bn1_g, bn1_b, bn2_g, bn2_b):
    emb_bf = (np.asarray(emb, dtype=np.float32) * SCALE).astype(ml_dtypes.bfloat16)

    def conv_lhst(w, K):
        # w: (1024, 512, K) -> [p=chan-in-chunk, ((k*2+cj)*8+ojj)*2+m, oj*128]
        a = np.transpose(np.asarray(w, np.float32), (2, 1, 0))        # (K, 512, 1024)
        a = a.reshape(K, 2, 2, P, 8, P)                               # (k,cj,m,p,ojj,o)
        a = a.transpose(3, 0, 1, 4, 2, 5)                             # (p,k,cj,ojj,m,o)
        a = np.ascontiguousarray(a.reshape(P, K * 32, P)) * SCALE
        return np.clip(a, -240.0, 240.0).astype(ml_dtypes.float8_e4m3)

    c3w = conv_lhst(conv3_w, 3)
    c5w = conv_lhst(conv5_w, 5)
    fcw = np.asarray(fc_w, np.float32).T.reshape(16, P, 8, P).transpose(1, 0, 2, 3)
    fcw = np.ascontiguousarray(fcw.reshape(P, P, P)).astype(ml_dtypes.bfloat16)
    clfw = np.ascontiguousarray(
        np.asarray(clf_w, np.float32).T.reshape(8, P, NCLS).transpose(1, 0, 2)
    ).astype(ml_dtypes.bfloat16)

    def conv_bias(b):
        # [P, 8]: cols 0..3 linear-branch bias (x SCALE2), cols 4..7 gate bias
        bb = np.ascontiguousarray(np.asarray(b, np.float32).reshape(8, P).T).copy()
        bb[:, 0:4] *= SCALE2
        return bb

    c3b = conv_bias(conv3_b)
    c5b = conv_bias(conv5_b)
    bn1g = np.ascontiguousarray(np.asarray(bn1_g, np.float32).reshape(16, P).T)
    bn1b = np.ascontiguousarray(np.asarray(bn1_b, np.float32).reshape(16, P).T)
    bn2g = np.ascontiguousarray(np.asarray(bn2_g, np.float32).reshape(8, P).T)
    bn2b = np.ascontiguousarray(np.asarray(bn2_b, np.float32).reshape(8, P).T)
    fcb = np.ascontiguousarray(np.asarray(fc_b, np.float32).reshape(8, P).T)
    clfb = np.zeros((P, 2), np.float32)
    clfb[:, 0] = np.asarray(clf_b, np.float32)[0:P]
    clfb[0:7, 1] = np.asarray(clf_b, np.float32)[P:NCLS]
    mask1_i = np.asarray(mask1).astype(np.int32)

    title = np.asarray(title).astype(np.int32)
    desc = np.asarray(desc).astype(np.int32)
    cate1 = np.asarray(cate1).astype(np.int32)

    in_maps = []
    for c in range(NCORES):
        sl = slice(c * BS, (c + 1) * BS)
        ti = _pack_tokens(title[sl], Lt, ST, WT_PAD)
        di = _pack_tokens(desc[sl], Ld, SD, WD_PAD)
        tok = np.ascontiguousarray(
            np.concatenate([ti, di]).reshape(NTOK, P).T
        )
        in_maps.append({
            "emb": emb_bf, "c3w": c3w, "c5w": c5w, "fcw": fcw, "clfw": clfw,
            "c3b": c3b, "c5b": c5b, "bn1g": bn1g, "bn1b": bn1b,
            "bn2g": bn2g, "bn2b": bn2b, "fcb": fcb, "clfb": clfb,
            "tok": tok, "cate1": cate1[sl].reshape(BS, 1), "mask1": mask1_i,
        })
    return in_maps


def kernel(**inputs) -> np.ndarray:
    nc = _get_nc()
    in_maps = make_in_maps(**inputs)
    res = run_bass_kernel_spmd(nc, in_maps, list(range(NCORES)))
    return np.concatenate([res.results[c]["out"] for c in range(NCORES)], axis=0)
